# revision 34
# baseline (speedup 1.0000x reference)
"""Multi-head self-attention Trainium2 kernel (8-core SPMD).

Problem: x[4,2048,1024] -> MHSA(16 heads, d=64) -> [4,2048,1024], f32.

Sharding: core = batch*2 + head_group. Each of the 8 cores handles one
batch (of 4) and one group of 8 heads (of 16): tensor-parallel over heads
x data-parallel over batch. The final FC contraction is split over head
groups; the host sums the two partial products per batch.

Math folds (exact up to float rounding):
 - b_k drops entirely: softmax over k is invariant to a per-q shift.
 - b_v contributes P@1 * b_v = b_v per row (softmax rows sum to 1), so
   b_v @ w_fc + b_fc is a constant [1024] vector added on the host.
 - b_q is folded into the Q^T projection via the scalar engine's free
   per-partition bias on the psum-evacuating activation copy.
 - The softmax denominator Z comes free from the PV matmul: V is stored
   with a ones column appended per head, so row 64 of the PV psum is Z.
 - V is scaled x32 on the host (fp8 range) and wfc by 1/32.

Precision: projections and scores run in bf16; the PV contraction runs
in fp8-e4m3 DoubleRow (two k-tiles = 256 keys per matmul, 2x PE rate).
P^T is written by the exp directly in e4m3; V_aug is e4m3.

Engine balance (all five engines are loaded in steady state):
 - TensorE: projections/FC (128-contraction), scores (64-contraction,
   two heads row-tiled concurrently via tile_position auto-derive), PV
   (fp8 DR). PV flushes are batched two pairs at a time to halve
   64<->128 tiling-mode switches.
 - ScalarE: table exp for 9/16 k-tiles per group + Q^T bias-add psum
   evacuation + PV-psum (po->posb) evacuation + half the FC stores.
 - VectorE: Schraudolph exp for 7/16 k-tiles (uint8 bits = round(A*s+B)
   IS the e4m3 encoding of ~exp(s)), K^T/V psum evacuation, batched
   reciprocal, half the FC stores.
 - GpSimd: 1/Z partition-broadcast + the normalize multiplies + the
   wv/x2/wfc weight DMAs (keeps them off the scalar/sync queues).
 - Sync: bulk x DMAs, y stores, zrow/OT row moves.

P^T layout is (ko, h, q) so the exp engines write contiguous [128,1024]
tiles; the DR moving operand picks the pair-interleave up via a strided
3D access pattern instead.
"""

import math
import numpy as np
from contextlib import ExitStack

import concourse.bass as bass
import concourse.tile as tile
import concourse.mybir as mybir
from concourse import bacc
from concourse._compat import with_exitstack
from concourse.bass_utils import run_bass_kernel_spmd

F32 = mybir.dt.float32
F32R = mybir.dt.float32r
BF16 = mybir.dt.bfloat16
FP8 = mybir.dt.float8e4
U8 = mybir.dt.uint8
DR = mybir.MatmulPerfMode.DoubleRow

# Schraudolph exp on the DVE: uint8 bits = round(A*s + B) ARE the e4m3
# encoding of ~exp(s) (3-mantissa-bit format, bias 7 -> 8*log2(p) + 56).
# B carries a -0.5 systematic-bias tweak (DVE converts round-to-nearest).
SCH_A = 8.0 / math.log(2.0)
SCH_B = 55.5
# k-tile PAIRS whose exp runs on the DVE (rest on ScalarE table exp).
# Pair-granular so each P^T tile is written by exactly one engine and the
# two engines' tile rings never couple through write-after-write ordering.
# The last head-pair's groups carry the FC psum evacuations on the DVE,
# so fewer exp pairs go there.
SCH_PAIRS = frozenset({1, 4, 6})
SCH_PAIRS_FC = frozenset({1, 4, 6})

B, S, E = 4, 2048, 1024
H, D = 16, 64
G = 2                      # head groups (tensor parallel)
HG = H // G                # 8 heads per core
DG = HG * D                # 512 = head-group width
NCORES = B * G             # 8

DT_X = BF16                # xT / wq / wk / wv / bq / ones (proj inputs)
DT_PROJ = BF16             # wfc / OT (FC operands)
DT_ATTN = BF16             # QT / KT / V_aug / PT

EC = E // 128              # 8  e-chunks
SC = S // 512              # 4  s-chunks (q-chunks)
ST = S // 128              # 16 s-tiles (k-tiles)
DTL = DG // 128            # 4  d-tiles (head pairs)
NT = E // 128              # 8  n-tiles of output

N_WARMUP = 10              # PE clock-gate warmup matmuls (DMA lead-in)


def _np_dt(dt):
    return np.dtype(mybir.dt.np(dt))


@with_exitstack
def _emit(ctx: ExitStack, tc: tile.TileContext, io: dict):
    nc = tc.nc
    xT_d, wq_d, wk_d, wv_d, bq_d, wfc_d, yT_d = (
        io["xT"], io["wq"], io["wk"], io["wv"], io["bq"], io["wfc"], io["yT"])

    sbW = ctx.enter_context(tc.tile_pool(name="sbW", bufs=1))
    sbP = ctx.enter_context(tc.tile_pool(name="sbP", bufs=1))
    xt_pool = ctx.enter_context(tc.tile_pool(name="xt", bufs=1))
    # separate P^T rings per exp engine (see SCH_PAIRS)
    pt_pool_s = ctx.enter_context(tc.tile_pool(name="pts", bufs=6))
    pt_pool_v = ctx.enter_context(tc.tile_pool(name="ptv", bufs=6))
    ev_pool = ctx.enter_context(tc.tile_pool(name="ev", bufs=6))
    nrm_pool = ctx.enter_context(tc.tile_pool(name="nrm", bufs=2))
    mm_ps = ctx.enter_context(tc.tile_pool(name="mmps", bufs=2, space="PSUM"))
    s_ps = ctx.enter_context(tc.tile_pool(name="sps", bufs=2, space="PSUM"))
    o_ps = ctx.enter_context(tc.tile_pool(name="ops", bufs=2, space="PSUM"))

    # ---- resident x^T (bf16, 4 MB, one [128, 8, 2048] tensor) + weights,
    # pre-cast to bf16 on the host. DMA queue plan (per-engine streams are
    # in-order, so issue order = need order):
    #   sync:   x0, bq, x1, x3   (x0 gates the first Q/K chain)
    #   scalar: wq, wk, wv, x2   (weights first; the x2 descriptor retires
    #                             before the first scalar compute is due)
    #   gpsimd: (wfc later, after pass A)
    xt_all = xt_pool.tile([128, EC, S], DT_X, name="xt", tag="xt")
    xTv = xT_d.rearrange("(ec p) s -> p ec s", p=128)
    wq_all = sbW.tile([128, EC, DG], DT_X, name="wq", tag="wq")
    wk_all = sbW.tile([128, EC, DG], DT_X, name="wk", tag="wk")
    wv_all = sbW.tile([128, EC, DG], DT_X, name="wv", tag="wv")
    def load_w(wt, wd, half, eng=None):
        (eng or nc.scalar).dma_start(
            wt[:, half * 4:(half + 1) * 4, :],
            wd.rearrange("(ec p) d -> p ec d",
                         p=128)[:, half * 4:(half + 1) * 4, :])

    # ---- PE warm-up: the HAM clock gate starts at half clock and only
    # ramps after ~3.4us of sustained matmul activity. Burn the DMA
    # lead-in (PE is idle anyway) on zero matmuls so the gate is warm
    # when real operands land. ----
    wu_sb = sbW.tile([128, 512], DT_X, name="wu", tag="wu")
    nc.vector.memset(wu_sb[:], 0.0)
    for i in range(N_WARMUP):
        wu_ps = o_ps.tile([65, 512], F32, name="wu_ps", tag="po")
        nc.tensor.matmul(wu_ps[:], wu_sb[:, 0:65], wu_sb[:],
                         start=True, stop=True)

    # Transfers within one queue serialize; across queues they share the
    # HBM bandwidth concurrently. Only sync + scalar are HWDGE (full
    # bandwidth) queues. Order pieces by first use, with x chunk 0 split
    # across both queues so the first Q/K chain unblocks earliest:
    #   sync:   x0[ec0-3], bq, x0[ec4-7], x1, x3, (wfc after pass A)
    #   scalar: wq, wk, wv, x2
    nc.sync.dma_start(xt_all[:, 0:4, 0:512], xTv[:, 0:4, 0:512])
    load_w(wq_all, wq_d, 0)
    load_w(wq_all, wq_d, 1)
    # bq as [128, 4]: column dt holds the 128 bias values of head-pair dt
    # (f32 so it can be the scalar-activation bias on the f32 psum)
    bq_t = sbW.tile([128, DTL], F32, name="bq", tag="bq")
    nc.sync.dma_start(bq_t[:], bq_d.rearrange("o (a p) -> (o p) a", p=128))
    nc.sync.dma_start(xt_all[:, 4:8, 0:512], xTv[:, 4:8, 0:512])
    load_w(wk_all, wk_d, 0)
    load_w(wk_all, wk_d, 1)
    load_w(wv_all, wv_d, 0)
    load_w(wv_all, wv_d, 1)
    nc.sync.dma_start(xt_all[:, :, 512:1024], xTv[:, :, 512:1024])
    nc.scalar.dma_start(xt_all[:, :, 1024:1536], xTv[:, :, 1024:1536])
    nc.sync.dma_start(xt_all[:, :, 1536:2048], xTv[:, :, 1536:2048])
    xt_res = [[xt_all[:, ec, sc * 512:(sc + 1) * 512] for sc in range(SC)]
              for ec in range(EC)]
    wq_t = [wq_all[:, ec, :] for ec in range(EC)]
    wk_t = [wk_all[:, ec, :] for ec in range(EC)]
    wv_t = [wv_all[:, ec, :] for ec in range(EC)]

    # ---- persistent activations (per-s-chunk tiles so attention's
    # dependencies are fine-grained and can overlap the projection) ----
    QT = [[sbP.tile([128, 512], DT_ATTN, name=f"QT{i}_{s}", tag=f"QT{i}_{s}")
           for s in range(SC)] for i in range(DTL)]
    KT = [[sbP.tile([128, 512], DT_ATTN, name=f"KT{i}_{s}", tag=f"KT{i}_{s}")
           for s in range(SC)] for i in range(DTL)]
    # V_aug in fp8 (values pre-scaled x32 on host; wfc carries 1/32), packed
    # as DoubleRow k-tile pairs: VA2[tp][:, ko, h*66+m] = V_aug for k-tile
    # 2*tp+ko, head h, column m (64 V cols + ones col at m=64; m=65 pad).
    # The ko stride (528) is a multiple of 16 as DoubleRow LDWEIGHTS needs.
    VA2 = [sbP.tile([128, 2, 8 * 66], FP8, name=f"VA2{i}", tag=f"VA2{i}")
           for i in range(ST // 2)]
    OT = [[sbP.tile([128, 512], DT_PROJ, name=f"OT{i}_{s}", tag=f"OT{i}_{s}")
           for s in range(SC)] for i in range(DTL)]

    # ones columns of V_aug (col 64 of each head's 66-col block)
    for tp in range(ST // 2):
        va4 = VA2[tp].rearrange("p ko (h c) -> p ko h c", c=66)
        nc.vector.memset(va4[:, :, :, 64:65], 1.0)

    # ---- projection pieces ----
    def emit_qk(dt_i, sc):
        """Q^T (with bias) and K^T for one head-pair tile, one s-chunk."""
        dsl = slice(dt_i * 128, (dt_i + 1) * 128)
        pq = mm_ps.tile([128, 512], F32, name="pq", tag="mm")
        for ec in range(EC):
            nc.tensor.matmul(pq[:], wq_t[ec][:, dsl], xt_res[ec][sc][:],
                             start=(ec == 0), stop=(ec == EC - 1))
        nc.vector.tensor_scalar_add(QT[dt_i][sc][:], pq[:],
                                    bq_t[:, dt_i:dt_i + 1])
        pk = mm_ps.tile([128, 512], F32, name="pk", tag="mm")
        for ec in range(EC):
            nc.tensor.matmul(pk[:], wk_t[ec][:, dsl], xt_res[ec][sc][:],
                             start=(ec == 0), stop=(ec == EC - 1))
        nc.vector.tensor_copy(KT[dt_i][sc][:], pk[:])

    def emit_v(sc):
        for st_l in range(4):
            st = sc * 4 + st_l
            ssl = slice(st_l * 128, (st_l + 1) * 128)
            pv = mm_ps.tile([128, 512], F32, name="pv", tag="mm")
            for ec in range(EC):
                nc.tensor.matmul(pv[:], xt_res[ec][sc][:, ssl], wv_t[ec][:],
                                 start=(ec == 0), stop=(ec == EC - 1))
            va4 = VA2[st // 2].rearrange("p ko (h c) -> p ko h c", c=66)
            pv3 = pv.rearrange("p (h d) -> p h d", d=64)
            # pass A only; ScalarE is idle there while the DVE carries
            # the K^T/Q^T evacuations and g0's Schraudolph tiles
            nc.scalar.copy(va4[:, st % 2, :, 0:64], pv3[:])

    def emit_fc(sc, tail=False):
        s0 = sc * 512
        for nt in range(NT):
            nsl = slice(nt * 128, (nt + 1) * 128)
            if tail and nt % 4 >= 2:
                # the score psums are idle in the tail: borrow them so
                # the FC accumulation ring is 4 deep instead of 2
                big = s_ps.tile([128, 1024], F32, name="pyb", tag="ps")
                py = big[:, 0:512]
            else:
                py = mm_ps.tile([128, 512], F32, name="py", tag="mm")[:]
            for dt_i in range(DTL):
                nc.tensor.matmul(py, wfc_t[dt_i][:, nsl],
                                 OT[dt_i][sc][:],
                                 start=(dt_i == 0), stop=(dt_i == DTL - 1))
            yv = ev_pool.tile([128, 512], F32, name="yv", tag="yv")
            # split the psum evacuation across both exp engines so the
            # psum ring recycles twice as fast
            if nt % 2 == 0:
                nc.scalar.copy(yv[:], py)
            else:
                nc.vector.tensor_copy(yv[:], py)
            nc.sync.dma_start(yT_d[nt * 128:(nt + 1) * 128, s0:s0 + 512],
                              yv[:])

    def attn_group(hp, qc):
        """Attention for one (head-pair, q-chunk) group, split-emittable.

        Returns (pairs, finish): pairs(lo, hi) emits S/exp/PV for k-tile
        pairs [lo, hi); finish() flushes the PV pipeline and normalizes.
        Callers must emit pairs in order and only after the KT/VA2 tiles
        for that k-range have been emitted (program-order dependency).
        """
        po = [o_ps.tile([65, 512], F32, name=f"po{p}", tag="po")
              for p in range(2)]

        def emit_pv(tp, pt2):
            # DoubleRow PV: one fp8 matmul per head covers k-tile pair
            # tp (256 keys). Stationary = VA2 pair slice [128, 2, 65];
            # moving = P^T pair [128, 2, 512] (ko-strided AP).
            pv4 = pt2.rearrange("p (ko h q) -> p h ko q", ko=2, h=2)
            va4 = VA2[tp].rearrange("p ko (h c) -> p ko h c", c=66)
            for p in range(2):
                h_l = hp * 2 + p
                nc.tensor.matmul(po[p][:],
                                 va4[:, :, h_l, 0:65],
                                 pv4[:, p],
                                 start=(tp == 0),
                                 stop=(tp == ST // 2 - 1),
                                 perf_mode=DR)

        # software-pipelined k-tile pairs: S/exp of pair tp are emitted
        # ahead of the PV of pair tp-3/-4; PVs are flushed two pairs at
        # a time so the 64<->128 tiling-mode switches halve.
        pend = []  # [(tp, pt2)] awaiting PV

        sch_pairs = SCH_PAIRS_FC if hp == DTL - 1 else SCH_PAIRS

        def pairs(lo, hi):
            for tp in range(lo, hi):
                # pt2 free-dim layout: ko*1024 + h*512 + q -> the exp
                # engines write contiguous [128, 1024] tiles per k-tile.
                sch = tp in sch_pairs
                pool = pt_pool_v if sch else pt_pool_s
                pt2 = pool.tile([128, 2048], FP8, name="ptt", tag="ptt")
                pt2v = pt2.rearrange("p (ko hq) -> p ko hq", ko=2)
                pt2u = pt2.bitcast(U8).rearrange("p (ko hq) -> p ko hq",
                                                 ko=2)
                for ko in range(2):
                    kt = 2 * tp + ko
                    ps_t = s_ps.tile([128, 1024], F32, name="ps", tag="ps")
                    for p in range(2):
                        psl = slice(p * 64, (p + 1) * 64)
                        nc.tensor.matmul(ps_t[:, p * 512:(p + 1) * 512],
                                         KT[hp][kt // 4][psl, (kt % 4) * 128:
                                                         (kt % 4) * 128 + 128],
                                         QT[hp][qc][psl, :],
                                         start=True, stop=True)
                    if sch:
                        nc.vector.tensor_scalar(pt2u[:, ko], ps_t[:],
                                                SCH_A, SCH_B,
                                                mybir.AluOpType.mult,
                                                mybir.AluOpType.add)
                    else:
                        nc.scalar.activation(pt2v[:, ko], ps_t[:],
                                             mybir.ActivationFunctionType.Exp)
                pend.append((tp, pt2))
                if len(pend) >= 4:
                    emit_pv(*pend.pop(0))
                    emit_pv(*pend.pop(0))

        def finish(tail=False):
            while pend:
                emit_pv(*pend.pop(0))
            _normalize(tail=tail)

        def _normalize(tail=False):
            # Evacuate both PV psums (releases the po banks), batch the
            # two heads' Z rows into one reciprocal, broadcast + multiply
            # on GpSimd. In the tail (nothing left to overlap) keep the
            # chain on the drained Scalar/Vector engines for min latency.
            # single [65, 1024] tile: head p in columns p*512:(p+1)*512,
            # evacuated by both exp engines in parallel, one Z-row DMA
            posb2 = nrm_pool.tile([65, 1024], F32, name="posb", tag="posb")
            posb = [posb2[:, 0:512], posb2[:, 512:1024]]
            # non-tail: both evacuations on the DVE — a posb copy in the
            # scalar stream sits behind the group's pending exps and
            # stalls the whole normalize chain at every group boundary
            if tail:
                nc.scalar.copy(posb[0], po[0][:])
            else:
                nc.vector.tensor_copy(posb[0], po[0][:])
            nc.vector.tensor_copy(posb[1], po[1][:])
            # custom DVE ops and partition_broadcast read the tensor's
            # partition 0 regardless of AP offset -> move Z via DMA first
            rz2 = nrm_pool.tile([1, 1024], F32, name="rz2", tag="rz2")
            nc.sync.dma_start(rz2[:], posb2[64:65, :])
            rzr = nrm_pool.tile([1, 1024], F32, name="rzr", tag="rzr")
            nc.vector.reciprocal_approx_fast(rzr[:], rz2[:])
            rzb = [nrm_pool.tile([64, 512], F32, name=f"rzb{p}",
                                 tag=f"rzb{p}") for p in range(2)]
            # broadcasts on GpSimd (its ONLY op class: switching between
            # custom and standard gpsimd ops reloads the Q7 library at
            # ~5.6us per switch); multiplies stay on the DVE
            for p in range(2):
                nc.gpsimd.partition_broadcast(
                    rzb[p][:], rzr[0:1, p * 512:(p + 1) * 512])
            tmp = nrm_pool.tile([64, 512], DT_PROJ, name="otmp", tag="otmp")
            nc.vector.tensor_mul(OT[hp][qc][0:64, :], posb[0][0:64, :],
                                 rzb[0][:])
            nc.vector.tensor_mul(tmp[:], posb[1][0:64, :], rzb[1][:])
            # engines cannot shift partitions; DMA moves rows 0:64 into
            # OT rows 64:128.
            nc.sync.dma_start(OT[hp][qc][64:128, :], tmp[:])

        return pairs, finish

    def emit_attn(hp, qc):
        pairs, finish = attn_group(hp, qc)
        pairs(0, ST // 2)
        finish()

    # ---- pass A: V (all heads) + Q/K for head-pair 0, with head-pair
    # 0's first attention group interleaved so the exp engines start as
    # soon as the first chunk's Q/K land (S/exp of pairs [lo,hi) only
    # needs KT; the PV of pair tp is emitted a few pairs later, by which
    # point the V chains covering it are in program order). ----
    g0_pairs, g0_finish = attn_group(0, 0)
    for sc in range(SC):
        emit_qk(0, sc)
        emit_v(sc)
        g0_pairs(2 * sc, 2 * sc + 2)
    g0_finish()
    # Deferred projections are spread one chain per attention group, so
    # each chain's psum evacuations sit between group workloads in the
    # (in-order) DVE queue instead of piling up behind a whole window.
    emit_qk(1, 0)
    for qc in range(1, SC):
        emit_attn(0, qc)
        emit_qk(1, qc)

    # wfc loads deferred past pass A: first FC use is in the hp3 window,
    # and these 2MB would otherwise crowd the DMA-paced startup. On the
    # sync queue they naturally sit behind the x chunks + g0's moves.
    wfc_t = []
    for dt_i in range(DTL):
        t = sbW.tile([128, E], DT_PROJ, name=f"wfc{dt_i}", tag=f"wfc{dt_i}")
        nc.sync.dma_start(t[:], wfc_d[dt_i * 128:(dt_i + 1) * 128, :])
        wfc_t.append(t)

    # ---- attention interleaved with deferred projections ----
    # Attention for head-pair hp runs while the projection for head-pair
    # hp+1 (emitted one chain per group, lower priority) fills PE gaps.
    # In the last head-pair, the FC of the previous q-chunk is emitted
    # into the middle of the group so it overlaps the exp-bound stretch
    # instead of piling up after the last group.
    for hp in range(1, DTL):
        for qc in range(SC):
            last = (hp == DTL - 1 and qc == SC - 1)
            pairs, finishg = attn_group(hp, qc)
            pairs(0, 4)
            if hp == DTL - 1 and qc >= 1:
                emit_fc(qc - 1)
            pairs(4, ST // 2)
            finishg(tail=last)
            if hp < DTL - 1:
                emit_qk(hp + 1, qc)
    emit_fc(SC - 1, tail=True)

_CACHE = {}


def _build():
    if "nc" in _CACHE:
        return _CACHE["nc"]
    nc = bacc.Bacc("TRN2", target_bir_lowering=False, debug=False)
    io = {
        "xT": nc.dram_tensor("xT", [E, S], BF16, kind="ExternalInput").ap(),
        "wq": nc.dram_tensor("wq", [E, DG], BF16, kind="ExternalInput").ap(),
        "wk": nc.dram_tensor("wk", [E, DG], BF16, kind="ExternalInput").ap(),
        "wv": nc.dram_tensor("wv", [E, DG], BF16, kind="ExternalInput").ap(),
        "bq": nc.dram_tensor("bq", [1, DG], F32, kind="ExternalInput").ap(),
        "wfc": nc.dram_tensor("wfc", [DG, E], BF16,
                              kind="ExternalInput").ap(),
        "yT": nc.dram_tensor("yT", [E, S], F32, kind="ExternalOutput").ap(),
    }
    with tile.TileContext(nc) as tc:
        _emit(tc, io)
    nc.compile()
    _CACHE["nc"] = nc
    return nc


def make_in_maps(x, w_qkv, b_qkv, w_fc):
    """Host-side sharding: returns per-core input dicts (core = b*G + g)."""
    import ml_dtypes
    x = np.asarray(x, dtype=np.float32)
    w_qkv = np.asarray(w_qkv, dtype=np.float32)
    b_qkv = np.asarray(b_qkv, dtype=np.float32)
    w_fc = np.asarray(w_fc, dtype=np.float32)
    npdt = ml_dtypes.bfloat16
    in_maps = []
    for b in range(B):
        xTb = np.ascontiguousarray(x[b].T).astype(npdt)
        for g in range(G):
            gs = slice(g * DG, (g + 1) * DG)
            in_maps.append({
                "xT": xTb,
                "wq": np.ascontiguousarray(
                    w_qkv[:, 0 * E:1 * E][:, gs] * (1.0 / np.sqrt(D))
                ).astype(npdt),
                "wk": np.ascontiguousarray(w_qkv[:, 1 * E:2 * E][:, gs]).astype(npdt),
                # V path scaled x32 so the fp8(e4m3) V_aug values sit in the
                # normal range; wfc carries the 1/32 back out.
                "wv": np.ascontiguousarray(
                    w_qkv[:, 2 * E:3 * E][:, gs] * 32.0).astype(npdt),
                "bq": np.ascontiguousarray(
                    b_qkv[0 * E:1 * E][gs][None, :] * (1.0 / np.sqrt(D))
                ).astype(np.float32),
                "wfc": np.ascontiguousarray(
                    w_fc[gs, :] * (1.0 / 32.0)).astype(npdt),
            })
    return in_maps


def gather(results, b_qkv, w_fc, b_fc):
    """Host-side unshard: sum group partials, transpose, add const bias."""
    b_qkv = np.asarray(b_qkv, dtype=np.float32)
    w_fc = np.asarray(w_fc, dtype=np.float32)
    b_fc = np.asarray(b_fc, dtype=np.float32)
    cbias = (b_qkv[2 * E:3 * E].astype(np.float64) @ w_fc.astype(np.float64)
             + b_fc.astype(np.float64)).astype(np.float32)
    y = np.empty((B, S, E), np.float32)
    for b in range(B):
        yT = results[b * G]["yT"] + results[b * G + 1]["yT"]
        y[b] = yT.T + cbias[None, :]
    return y


def kernel(x, w_qkv, b_qkv, w_fc, b_fc, _trace=False, _tmpdir=None):
    nc = _build()
    in_maps = make_in_maps(x, w_qkv, b_qkv, w_fc)
    res = run_bass_kernel_spmd(nc, in_maps, list(range(NCORES)),
                               trace=_trace, tmpdir=_tmpdir)
    y = gather(res.results, b_qkv, w_fc, b_fc)
    kernel.last_exec_time_ns = res.exec_time_ns
    kernel.last_res = res
    return y


# revision 36
# speedup vs baseline: 1.1545x; 1.1545x over previous
"""Multi-head self-attention Trainium2 kernel (8-core SPMD).

Problem: x[4,2048,1024] -> MHSA(16 heads, d=64) -> [4,2048,1024], f32.

Sharding: core = batch*2 + head_group. Each of the 8 cores handles one
batch (of 4) and one group of 8 heads (of 16): tensor-parallel over heads
x data-parallel over batch. The final FC contraction is split over head
groups; the host sums the two partial products per batch.

Math folds (exact up to float rounding):
 - b_k drops entirely: softmax over k is invariant to a per-q shift.
 - b_v contributes P@1 * b_v = b_v per row (softmax rows sum to 1), so
   b_v @ w_fc + b_fc is a constant [1024] vector added on the host.
 - b_q is folded into the Q^T projection via the scalar engine's free
   per-partition bias on the psum-evacuating activation copy.
 - The softmax denominator Z comes free from the PV matmul: V is stored
   with a ones column appended per head, so row 64 of the PV psum is Z.
 - V is scaled x32 on the host (fp8 range) and wfc by 1/32.

Precision: projections and scores run in bf16; the PV contraction runs
in fp8-e4m3 DoubleRow (two k-tiles = 256 keys per matmul, 2x PE rate).
P^T is written by the exp directly in e4m3; V_aug is e4m3.

Engine balance (all five engines are loaded in steady state):
 - TensorE: projections/FC (128-contraction), scores (64-contraction,
   two heads row-tiled concurrently via tile_position auto-derive), PV
   (fp8 DR). PV flushes are batched two pairs at a time to halve
   64<->128 tiling-mode switches.
 - ScalarE: table exp for 9/16 k-tiles per group + Q^T bias-add psum
   evacuation + PV-psum (po->posb) evacuation + half the FC stores.
 - VectorE: Schraudolph exp for 7/16 k-tiles (uint8 bits = round(A*s+B)
   IS the e4m3 encoding of ~exp(s)), K^T/V psum evacuation, batched
   reciprocal, half the FC stores.
 - GpSimd: 1/Z partition-broadcast + the normalize multiplies + the
   wv/x2/wfc weight DMAs (keeps them off the scalar/sync queues).
 - Sync: bulk x DMAs, y stores, zrow/OT row moves.

P^T layout is (ko, h, q) so the exp engines write contiguous [128,1024]
tiles; the DR moving operand picks the pair-interleave up via a strided
3D access pattern instead.
"""

import math
import numpy as np
from contextlib import ExitStack

import concourse.bass as bass
import concourse.tile as tile
import concourse.mybir as mybir
from concourse import bacc
from concourse._compat import with_exitstack
from concourse.bass_utils import run_bass_kernel_spmd

F32 = mybir.dt.float32
F32R = mybir.dt.float32r
BF16 = mybir.dt.bfloat16
FP8 = mybir.dt.float8e4
U8 = mybir.dt.uint8
DR = mybir.MatmulPerfMode.DoubleRow

# Schraudolph exp on the DVE: uint8 bits = round(A*s + B) ARE the e4m3
# encoding of ~exp(s) (3-mantissa-bit format, bias 7 -> 8*log2(p) + 56).
# B carries a -0.5 systematic-bias tweak (DVE converts round-to-nearest).
SCH_A = 8.0 / math.log(2.0)
SCH_B = 55.5
# k-tile PAIRS whose exp runs on the DVE (rest on ScalarE table exp).
# Pair-granular so each P^T tile is written by exactly one engine and the
# two engines' tile rings never couple through write-after-write ordering.
# The last head-pair's groups carry the FC psum evacuations on the DVE,
# so fewer exp pairs go there.
SCH_PAIRS = frozenset({1, 4, 6})
SCH_PAIRS_FC = frozenset({1, 4, 6})

B, S, E = 4, 2048, 1024
H, D = 16, 64
G = 2                      # head groups (tensor parallel)
HG = H // G                # 8 heads per core
DG = HG * D                # 512 = head-group width
NCORES = B * G             # 8

DT_X = BF16                # xT / wq / wk / wv / bq / ones (proj inputs)
DT_PROJ = BF16             # wfc / OT (FC operands)
DT_ATTN = BF16             # QT / KT / V_aug / PT

EC = E // 128              # 8  e-chunks
SC = S // 512              # 4  s-chunks (q-chunks)
ST = S // 128              # 16 s-tiles (k-tiles)
DTL = DG // 128            # 4  d-tiles (head pairs)
NT = E // 128              # 8  n-tiles of output

N_WARMUP = 10              # PE clock-gate warmup matmuls (DMA lead-in)


def _np_dt(dt):
    return np.dtype(mybir.dt.np(dt))


@with_exitstack
def _emit(ctx: ExitStack, tc: tile.TileContext, io: dict):
    nc = tc.nc
    xT_d, wq_d, wk_d, wv_d, bq_d, wfc_d, yT_d = (
        io["xT"], io["wq"], io["wk"], io["wv"], io["bq"], io["wfc"], io["yT"])

    sbW = ctx.enter_context(tc.tile_pool(name="sbW", bufs=1))
    sbP = ctx.enter_context(tc.tile_pool(name="sbP", bufs=1))
    xt_pool = ctx.enter_context(tc.tile_pool(name="xt", bufs=1))
    # separate P^T rings per exp engine (see SCH_PAIRS)
    pt_pool_s = ctx.enter_context(tc.tile_pool(name="pts", bufs=6))
    pt_pool_v = ctx.enter_context(tc.tile_pool(name="ptv", bufs=6))
    ev_pool = ctx.enter_context(tc.tile_pool(name="ev", bufs=6))
    nrm_pool = ctx.enter_context(tc.tile_pool(name="nrm", bufs=2))
    mm_ps = ctx.enter_context(tc.tile_pool(name="mmps", bufs=2, space="PSUM"))
    s_ps = ctx.enter_context(tc.tile_pool(name="sps", bufs=2, space="PSUM"))
    o_ps = ctx.enter_context(tc.tile_pool(name="ops", bufs=2, space="PSUM"))

    # ---- resident x^T (bf16, 4 MB, one [128, 8, 2048] tensor) + weights,
    # pre-cast to bf16 on the host. DMA queue plan (per-engine streams are
    # in-order, so issue order = need order):
    #   sync:   x0, bq, x1, x3   (x0 gates the first Q/K chain)
    #   scalar: wq, wk, wv, x2   (weights first; the x2 descriptor retires
    #                             before the first scalar compute is due)
    #   gpsimd: (wfc later, after pass A)
    xt_all = xt_pool.tile([128, EC, S], DT_X, name="xt", tag="xt")
    xTv = xT_d.rearrange("(ec p) s -> p ec s", p=128)
    wq_all = sbW.tile([128, EC, DG], DT_X, name="wq", tag="wq")
    wk_all = sbW.tile([128, EC, DG], DT_X, name="wk", tag="wk")
    wv_all = sbW.tile([128, EC, DG], DT_X, name="wv", tag="wv")
    def load_w(wt, wd, half, eng=None):
        (eng or nc.scalar).dma_start(
            wt[:, half * 4:(half + 1) * 4, :],
            wd.rearrange("(ec p) d -> p ec d",
                         p=128)[:, half * 4:(half + 1) * 4, :])

    # ---- PE warm-up: the HAM clock gate starts at half clock and only
    # ramps after ~3.4us of sustained matmul activity. Burn the DMA
    # lead-in (PE is idle anyway) on zero matmuls so the gate is warm
    # when real operands land. ----
    wu_sb = sbW.tile([128, 512], DT_X, name="wu", tag="wu")
    nc.vector.memset(wu_sb[:], 0.0)
    for i in range(N_WARMUP):
        wu_ps = o_ps.tile([65, 512], F32, name="wu_ps", tag="po")
        nc.tensor.matmul(wu_ps[:], wu_sb[:, 0:65], wu_sb[:],
                         start=True, stop=True)

    # Transfers within one queue serialize; across queues they share the
    # HBM bandwidth concurrently. Only sync + scalar are HWDGE (full
    # bandwidth) queues. Order pieces by first use, with x chunk 0 split
    # across both queues so the first Q/K chain unblocks earliest:
    #   sync:   x0[ec0-3], bq, x0[ec4-7], x1, x3, (wfc after pass A)
    #   scalar: wq, wk, wv, x2
    nc.sync.dma_start(xt_all[:, 0:4, 0:512], xTv[:, 0:4, 0:512])
    load_w(wq_all, wq_d, 0)
    load_w(wq_all, wq_d, 1)
    # bq as [128, 4]: column dt holds the 128 bias values of head-pair dt
    # (f32 so it can be the scalar-activation bias on the f32 psum)
    bq_t = sbW.tile([128, DTL], F32, name="bq", tag="bq")
    nc.sync.dma_start(bq_t[:], bq_d.rearrange("o (a p) -> (o p) a", p=128))
    nc.sync.dma_start(xt_all[:, 4:8, 0:512], xTv[:, 4:8, 0:512])
    load_w(wk_all, wk_d, 0)
    load_w(wk_all, wk_d, 1)
    load_w(wv_all, wv_d, 0)
    load_w(wv_all, wv_d, 1)
    # x1 behind the weights on scalar (its descriptor retires before the
    # first scalar compute is due); x2/x3 on sync, whose first real op
    # (g0's Z-row move) is far out
    nc.scalar.dma_start(xt_all[:, :, 512:1024], xTv[:, :, 512:1024])
    nc.sync.dma_start(xt_all[:, :, 1024:1536], xTv[:, :, 1024:1536])
    nc.sync.dma_start(xt_all[:, :, 1536:2048], xTv[:, :, 1536:2048])
    xt_res = [[xt_all[:, ec, sc * 512:(sc + 1) * 512] for sc in range(SC)]
              for ec in range(EC)]
    wq_t = [wq_all[:, ec, :] for ec in range(EC)]
    wk_t = [wk_all[:, ec, :] for ec in range(EC)]
    wv_t = [wv_all[:, ec, :] for ec in range(EC)]

    # ---- persistent activations (per-s-chunk tiles so attention's
    # dependencies are fine-grained and can overlap the projection) ----
    QT = [[sbP.tile([128, 512], DT_ATTN, name=f"QT{i}_{s}", tag=f"QT{i}_{s}")
           for s in range(SC)] for i in range(DTL)]
    KT = [[sbP.tile([128, 512], DT_ATTN, name=f"KT{i}_{s}", tag=f"KT{i}_{s}")
           for s in range(SC)] for i in range(DTL)]
    # V_aug in fp8 (values pre-scaled x32 on host; wfc carries 1/32), packed
    # as DoubleRow k-tile pairs: VA2[tp][:, ko, h*66+m] = V_aug for k-tile
    # 2*tp+ko, head h, column m (64 V cols + ones col at m=64; m=65 pad).
    # The ko stride (528) is a multiple of 16 as DoubleRow LDWEIGHTS needs.
    VA2 = [sbP.tile([128, 2, 8 * 66], FP8, name=f"VA2{i}", tag=f"VA2{i}")
           for i in range(ST // 2)]
    OT = [[sbP.tile([128, 512], DT_PROJ, name=f"OT{i}_{s}", tag=f"OT{i}_{s}")
           for s in range(SC)] for i in range(DTL)]

    # ones columns of V_aug (col 64 of each head's 66-col block)
    for tp in range(ST // 2):
        va4 = VA2[tp].rearrange("p ko (h c) -> p ko h c", c=66)
        nc.vector.memset(va4[:, :, :, 64:65], 1.0)

    # ---- projection pieces ----
    def emit_qk(dt_i, sc):
        """Q^T (with bias) and K^T for one head-pair tile, one s-chunk."""
        dsl = slice(dt_i * 128, (dt_i + 1) * 128)
        pq = mm_ps.tile([128, 512], F32, name="pq", tag="mm")
        for ec in range(EC):
            nc.tensor.matmul(pq[:], wq_t[ec][:, dsl], xt_res[ec][sc][:],
                             start=(ec == 0), stop=(ec == EC - 1))
        nc.vector.tensor_scalar_add(QT[dt_i][sc][:], pq[:],
                                    bq_t[:, dt_i:dt_i + 1])
        pk = mm_ps.tile([128, 512], F32, name="pk", tag="mm")
        for ec in range(EC):
            nc.tensor.matmul(pk[:], wk_t[ec][:, dsl], xt_res[ec][sc][:],
                             start=(ec == 0), stop=(ec == EC - 1))
        nc.vector.tensor_copy(KT[dt_i][sc][:], pk[:])

    def emit_v(sc):
        for st_l in range(4):
            st = sc * 4 + st_l
            ssl = slice(st_l * 128, (st_l + 1) * 128)
            pv = mm_ps.tile([128, 512], F32, name="pv", tag="mm")
            for ec in range(EC):
                nc.tensor.matmul(pv[:], xt_res[ec][sc][:, ssl], wv_t[ec][:],
                                 start=(ec == 0), stop=(ec == EC - 1))
            va4 = VA2[st // 2].rearrange("p ko (h c) -> p ko h c", c=66)
            pv3 = pv.rearrange("p (h d) -> p h d", d=64)
            # pass A only; ScalarE is idle there while the DVE carries
            # the K^T/Q^T evacuations and g0's Schraudolph tiles
            nc.scalar.copy(va4[:, st % 2, :, 0:64], pv3[:])

    def emit_fc(sc, tail=False):
        s0 = sc * 512
        for nt in range(NT):
            nsl = slice(nt * 128, (nt + 1) * 128)
            if tail and nt % 4 >= 2:
                # the score psums are idle in the tail: borrow them so
                # the FC accumulation ring is 4 deep instead of 2
                big = s_ps.tile([128, 1024], F32, name="pyb", tag="ps")
                py = big[:, 0:512]
            else:
                py = mm_ps.tile([128, 512], F32, name="py", tag="mm")[:]
            for dt_i in range(DTL):
                nc.tensor.matmul(py, wfc_t[dt_i][:, nsl],
                                 OT[dt_i][sc][:],
                                 start=(dt_i == 0), stop=(dt_i == DTL - 1))
            yv = ev_pool.tile([128, 512], F32, name="yv", tag="yv")
            # split the psum evacuation across both exp engines so the
            # psum ring recycles twice as fast
            if nt % 2 == 0:
                nc.scalar.copy(yv[:], py)
            else:
                nc.vector.tensor_copy(yv[:], py)
            nc.sync.dma_start(yT_d[nt * 128:(nt + 1) * 128, s0:s0 + 512],
                              yv[:])

    def attn_group(hp, qc):
        """Attention for one (head-pair, q-chunk) group, split-emittable.

        Returns (pairs, finish): pairs(lo, hi) emits S/exp/PV for k-tile
        pairs [lo, hi); finish() flushes the PV pipeline and normalizes.
        Callers must emit pairs in order and only after the KT/VA2 tiles
        for that k-range have been emitted (program-order dependency).
        """
        po = [o_ps.tile([65, 512], F32, name=f"po{p}", tag="po")
              for p in range(2)]

        def emit_pv(tp, pt2):
            # DoubleRow PV: one fp8 matmul per head covers k-tile pair
            # tp (256 keys). Stationary = VA2 pair slice [128, 2, 65];
            # moving = P^T pair [128, 2, 512] (ko-strided AP).
            pv4 = pt2.rearrange("p (ko h q) -> p h ko q", ko=2, h=2)
            va4 = VA2[tp].rearrange("p ko (h c) -> p ko h c", c=66)
            for p in range(2):
                h_l = hp * 2 + p
                nc.tensor.matmul(po[p][:],
                                 va4[:, :, h_l, 0:65],
                                 pv4[:, p],
                                 start=(tp == 0),
                                 stop=(tp == ST // 2 - 1),
                                 perf_mode=DR)

        # software-pipelined k-tile pairs: S/exp of pair tp are emitted
        # ahead of the PV of pair tp-3/-4; PVs are flushed two pairs at
        # a time so the 64<->128 tiling-mode switches halve.
        pend = []  # [(tp, pt2)] awaiting PV

        sch_pairs = SCH_PAIRS_FC if hp == DTL - 1 else SCH_PAIRS

        def pairs(lo, hi):
            for tp in range(lo, hi):
                # pt2 free-dim layout: ko*1024 + h*512 + q -> the exp
                # engines write contiguous [128, 1024] tiles per k-tile.
                sch = tp in sch_pairs
                pool = pt_pool_v if sch else pt_pool_s
                pt2 = pool.tile([128, 2048], FP8, name="ptt", tag="ptt")
                pt2v = pt2.rearrange("p (ko hq) -> p ko hq", ko=2)
                pt2u = pt2.bitcast(U8).rearrange("p (ko hq) -> p ko hq",
                                                 ko=2)
                for ko in range(2):
                    kt = 2 * tp + ko
                    ps_t = s_ps.tile([128, 1024], F32, name="ps", tag="ps")
                    for p in range(2):
                        psl = slice(p * 64, (p + 1) * 64)
                        nc.tensor.matmul(ps_t[:, p * 512:(p + 1) * 512],
                                         KT[hp][kt // 4][psl, (kt % 4) * 128:
                                                         (kt % 4) * 128 + 128],
                                         QT[hp][qc][psl, :],
                                         start=True, stop=True)
                    if sch:
                        nc.vector.tensor_scalar(pt2u[:, ko], ps_t[:],
                                                SCH_A, SCH_B,
                                                mybir.AluOpType.mult,
                                                mybir.AluOpType.add)
                    else:
                        nc.scalar.activation(pt2v[:, ko], ps_t[:],
                                             mybir.ActivationFunctionType.Exp)
                pend.append((tp, pt2))
                if len(pend) >= 4:
                    emit_pv(*pend.pop(0))
                    emit_pv(*pend.pop(0))

        def finish(tail=False):
            while pend:
                emit_pv(*pend.pop(0))
            _normalize(tail=tail)

        def _normalize(tail=False):
            # Evacuate both PV psums (releases the po banks), batch the
            # two heads' Z rows into one reciprocal, broadcast + multiply
            # on GpSimd. In the tail (nothing left to overlap) keep the
            # chain on the drained Scalar/Vector engines for min latency.
            # single [65, 1024] tile: head p in columns p*512:(p+1)*512,
            # evacuated by both exp engines in parallel, one Z-row DMA
            posb2 = nrm_pool.tile([65, 1024], F32, name="posb", tag="posb")
            posb = [posb2[:, 0:512], posb2[:, 512:1024]]
            # non-tail: both evacuations on the DVE — a posb copy in the
            # scalar stream sits behind the group's pending exps and
            # stalls the whole normalize chain at every group boundary
            if tail:
                nc.scalar.copy(posb[0], po[0][:])
            else:
                nc.vector.tensor_copy(posb[0], po[0][:])
            nc.vector.tensor_copy(posb[1], po[1][:])
            # custom DVE ops and partition_broadcast read the tensor's
            # partition 0 regardless of AP offset -> move Z via DMA first
            rz2 = nrm_pool.tile([1, 1024], F32, name="rz2", tag="rz2")
            nc.sync.dma_start(rz2[:], posb2[64:65, :])
            rzr = nrm_pool.tile([1, 1024], F32, name="rzr", tag="rzr")
            nc.vector.reciprocal_approx_fast(rzr[:], rz2[:])
            rzb = [nrm_pool.tile([64, 512], F32, name=f"rzb{p}",
                                 tag=f"rzb{p}") for p in range(2)]
            # broadcasts on GpSimd (its ONLY op class: switching between
            # custom and standard gpsimd ops reloads the Q7 library at
            # ~5.6us per switch); multiplies stay on the DVE
            for p in range(2):
                nc.gpsimd.partition_broadcast(
                    rzb[p][:], rzr[0:1, p * 512:(p + 1) * 512])
            tmp = nrm_pool.tile([64, 512], DT_PROJ, name="otmp", tag="otmp")
            nc.vector.tensor_mul(OT[hp][qc][0:64, :], posb[0][0:64, :],
                                 rzb[0][:])
            nc.vector.tensor_mul(tmp[:], posb[1][0:64, :], rzb[1][:])
            # engines cannot shift partitions; DMA moves rows 0:64 into
            # OT rows 64:128.
            nc.sync.dma_start(OT[hp][qc][64:128, :], tmp[:])

        return pairs, finish

    def emit_attn(hp, qc):
        pairs, finish = attn_group(hp, qc)
        pairs(0, ST // 2)
        finish()

    # ---- pass A: V (all heads) + Q/K for head-pair 0, with head-pair
    # 0's first attention group interleaved so the exp engines start as
    # soon as the first chunk's Q/K land (S/exp of pairs [lo,hi) only
    # needs KT; the PV of pair tp is emitted a few pairs later, by which
    # point the V chains covering it are in program order). ----
    g0_pairs, g0_finish = attn_group(0, 0)
    for sc in range(SC):
        emit_qk(0, sc)
        emit_v(sc)
        g0_pairs(2 * sc, 2 * sc + 2)
    emit_qk(1, 0)

    # ---- attention groups, score-primed across boundaries ----
    # Each group's first two score pairs are emitted BEFORE the previous
    # group's PV flush + normalize, so the exp pipeline never drains at
    # a group boundary. Deferred projections are spread one chain per
    # group (their psum evacuations then sit between group workloads in
    # the in-order DVE queue instead of piling up behind a window); the
    # hp3 groups carry the FC of the previous q-chunk instead.
    wfc_t = []
    prev_fin = g0_finish
    seq = [(0, qc) for qc in range(1, SC)] + \
          [(hp, qc) for hp in range(1, DTL) for qc in range(SC)]
    for hp, qc in seq:
        pairs, fin = attn_group(hp, qc)
        pairs(0, 2)
        prev_fin()
        pairs(2, 4)
        if hp == DTL - 1 and qc >= 1:
            emit_fc(qc - 1)
        pairs(4, ST // 2)
        if hp < DTL - 1 and qc >= (1 if hp == 0 else 0):
            emit_qk(hp + 1, qc)
        prev_fin = fin
        if (hp, qc) == (0, SC - 1):
            # wfc deferred past pass A: first FC use is in the hp3
            # window, and these 2MB would otherwise crowd the DMA-paced
            # startup.
            for dt_i in range(DTL):
                t = sbW.tile([128, E], DT_PROJ, name=f"wfc{dt_i}",
                             tag=f"wfc{dt_i}")
                nc.sync.dma_start(t[:], wfc_d[dt_i * 128:(dt_i + 1) * 128, :])
                wfc_t.append(t)
    prev_fin(tail=True)
    emit_fc(SC - 1, tail=True)

_CACHE = {}


def _build():
    if "nc" in _CACHE:
        return _CACHE["nc"]
    nc = bacc.Bacc("TRN2", target_bir_lowering=False, debug=False)
    io = {
        "xT": nc.dram_tensor("xT", [E, S], BF16, kind="ExternalInput").ap(),
        "wq": nc.dram_tensor("wq", [E, DG], BF16, kind="ExternalInput").ap(),
        "wk": nc.dram_tensor("wk", [E, DG], BF16, kind="ExternalInput").ap(),
        "wv": nc.dram_tensor("wv", [E, DG], BF16, kind="ExternalInput").ap(),
        "bq": nc.dram_tensor("bq", [1, DG], F32, kind="ExternalInput").ap(),
        "wfc": nc.dram_tensor("wfc", [DG, E], BF16,
                              kind="ExternalInput").ap(),
        "yT": nc.dram_tensor("yT", [E, S], F32, kind="ExternalOutput").ap(),
    }
    with tile.TileContext(nc) as tc:
        _emit(tc, io)
    nc.compile()
    _CACHE["nc"] = nc
    return nc


def make_in_maps(x, w_qkv, b_qkv, w_fc):
    """Host-side sharding: returns per-core input dicts (core = b*G + g)."""
    import ml_dtypes
    x = np.asarray(x, dtype=np.float32)
    w_qkv = np.asarray(w_qkv, dtype=np.float32)
    b_qkv = np.asarray(b_qkv, dtype=np.float32)
    w_fc = np.asarray(w_fc, dtype=np.float32)
    npdt = ml_dtypes.bfloat16
    in_maps = []
    for b in range(B):
        xTb = np.ascontiguousarray(x[b].T).astype(npdt)
        for g in range(G):
            gs = slice(g * DG, (g + 1) * DG)
            in_maps.append({
                "xT": xTb,
                "wq": np.ascontiguousarray(
                    w_qkv[:, 0 * E:1 * E][:, gs] * (1.0 / np.sqrt(D))
                ).astype(npdt),
                "wk": np.ascontiguousarray(w_qkv[:, 1 * E:2 * E][:, gs]).astype(npdt),
                # V path scaled x32 so the fp8(e4m3) V_aug values sit in the
                # normal range; wfc carries the 1/32 back out.
                "wv": np.ascontiguousarray(
                    w_qkv[:, 2 * E:3 * E][:, gs] * 32.0).astype(npdt),
                "bq": np.ascontiguousarray(
                    b_qkv[0 * E:1 * E][gs][None, :] * (1.0 / np.sqrt(D))
                ).astype(np.float32),
                "wfc": np.ascontiguousarray(
                    w_fc[gs, :] * (1.0 / 32.0)).astype(npdt),
            })
    return in_maps


def gather(results, b_qkv, w_fc, b_fc):
    """Host-side unshard: sum group partials, transpose, add const bias."""
    b_qkv = np.asarray(b_qkv, dtype=np.float32)
    w_fc = np.asarray(w_fc, dtype=np.float32)
    b_fc = np.asarray(b_fc, dtype=np.float32)
    cbias = (b_qkv[2 * E:3 * E].astype(np.float64) @ w_fc.astype(np.float64)
             + b_fc.astype(np.float64)).astype(np.float32)
    y = np.empty((B, S, E), np.float32)
    for b in range(B):
        yT = results[b * G]["yT"] + results[b * G + 1]["yT"]
        y[b] = yT.T + cbias[None, :]
    return y


def kernel(x, w_qkv, b_qkv, w_fc, b_fc, _trace=False, _tmpdir=None):
    nc = _build()
    in_maps = make_in_maps(x, w_qkv, b_qkv, w_fc)
    res = run_bass_kernel_spmd(nc, in_maps, list(range(NCORES)),
                               trace=_trace, tmpdir=_tmpdir)
    y = gather(res.results, b_qkv, w_fc, b_fc)
    kernel.last_exec_time_ns = res.exec_time_ns
    kernel.last_res = res
    return y


# revision 44
# speedup vs baseline: 1.1635x; 1.0078x over previous
"""Multi-head self-attention Trainium2 kernel (8-core SPMD).

Problem: x[4,2048,1024] -> MHSA(16 heads, d=64) -> [4,2048,1024], f32.

Sharding: core = batch*2 + head_group. Each of the 8 cores handles one
batch (of 4) and one group of 8 heads (of 16): tensor-parallel over heads
x data-parallel over batch. The final FC contraction is split over head
groups; the host sums the two partial products per batch.

Math folds (exact up to float rounding):
 - b_k drops entirely: softmax over k is invariant to a per-q shift.
 - b_v contributes P@1 * b_v = b_v per row (softmax rows sum to 1), so
   b_v @ w_fc + b_fc is a constant [1024] vector added on the host.
 - b_q is folded into the Q^T projection via the scalar engine's free
   per-partition bias on the psum-evacuating activation copy.
 - The softmax denominator Z comes free from the PV matmul: V is stored
   with a ones column appended per head, so row 64 of the PV psum is Z.
 - V is scaled x32 on the host (fp8 range) and wfc by 1/32.

Precision: projections and scores run in bf16; the PV contraction runs
in fp8-e4m3 DoubleRow (two k-tiles = 256 keys per matmul, 2x PE rate).
P^T is written by the exp directly in e4m3; V_aug is e4m3.

Engine balance (all five engines are loaded in steady state). The
engine queues are IN-ORDER, so placement is chosen to avoid
head-of-line blocking between the exp streams and psum evacuations:
 - TensorE: projections/FC (128-contraction), scores (64-contraction,
   two heads row-tiled concurrently via tile_position auto-derive), PV
   (fp8 DR). PV flushes are batched two pairs at a time to halve
   64<->128 tiling-mode switches.
 - ScalarE: table exp for 10/16 k-tiles per group (pair-granular, own
   P^T tile ring) + the pass-A V evacuations + half the FC stores.
 - VectorE: Schraudolph exp for 6/16 k-tiles (uint8 bits = round(A*s+B)
   IS the e4m3 encoding of ~exp(s); own P^T ring), Q^T/K^T/PV psum
   evacuation, batched reciprocal, normalize multiplies, FC stores.
 - GpSimd: ONLY the 1/Z partition-broadcast (mixing gpsimd op classes
   reloads the Q7 library at ~5.6us per switch); the tail normalize
   broadcasts instead run as ones[1,64].T @ rz matmuls on the idle PE.
 - Sync/Scalar DMA (the two HWDGE queues): weights+x ordered by first
   use with x chunk 0 split across both; y stores, zrow/OT row moves.

P^T layout is (ko, h, q) so the exp engines write contiguous [128,1024]
tiles; the DR moving operand picks the pair-interleave up via a strided
3D access pattern instead. Every attention group's first two score
pairs are emitted before the previous group's PV flush + normalize
("score priming"), which removes all group-boundary PE bubbles.
"""

import math
import numpy as np
from contextlib import ExitStack

import concourse.bass as bass
import concourse.tile as tile
import concourse.mybir as mybir
from concourse import bacc
from concourse._compat import with_exitstack
from concourse.bass_utils import run_bass_kernel_spmd

F32 = mybir.dt.float32
F32R = mybir.dt.float32r
BF16 = mybir.dt.bfloat16
FP8 = mybir.dt.float8e4
U8 = mybir.dt.uint8
DR = mybir.MatmulPerfMode.DoubleRow

# Schraudolph exp on the DVE: uint8 bits = round(A*s + B) ARE the e4m3
# encoding of ~exp(s) (3-mantissa-bit format, bias 7 -> 8*log2(p) + 56).
# B carries a -0.5 systematic-bias tweak (DVE converts round-to-nearest).
SCH_A = 8.0 / math.log(2.0)
SCH_B = 55.5
# k-tile PAIRS whose exp runs on the DVE (rest on ScalarE table exp).
# Pair-granular so each P^T tile is written by exactly one engine and the
# two engines' tile rings never couple through write-after-write ordering.
# The last head-pair's groups carry the FC psum evacuations on the DVE,
# so fewer exp pairs go there.
SCH_PAIRS = frozenset({1, 4, 6})
SCH_PAIRS_FC = frozenset({1, 4, 6})

B, S, E = 4, 2048, 1024
H, D = 16, 64
G = 2                      # head groups (tensor parallel)
HG = H // G                # 8 heads per core
DG = HG * D                # 512 = head-group width
NCORES = B * G             # 8

DT_X = BF16                # xT / wq / wk / wv / bq / ones (proj inputs)
DT_PROJ = BF16             # wfc / OT (FC operands)
DT_ATTN = BF16             # QT / KT / V_aug / PT

EC = E // 128              # 8  e-chunks
SC = S // 512              # 4  s-chunks (q-chunks)
ST = S // 128              # 16 s-tiles (k-tiles)
DTL = DG // 128            # 4  d-tiles (head pairs)
NT = E // 128              # 8  n-tiles of output

N_WARMUP = 10              # PE clock-gate warmup matmuls (DMA lead-in)


def _np_dt(dt):
    return np.dtype(mybir.dt.np(dt))


@with_exitstack
def _emit(ctx: ExitStack, tc: tile.TileContext, io: dict):
    nc = tc.nc
    xT_d, wq_d, wk_d, wv_d, bq_d, wfc_d, yT_d = (
        io["xT"], io["wq"], io["wk"], io["wv"], io["bq"], io["wfc"], io["yT"])

    sbW = ctx.enter_context(tc.tile_pool(name="sbW", bufs=1))
    sbP = ctx.enter_context(tc.tile_pool(name="sbP", bufs=1))
    xt_pool = ctx.enter_context(tc.tile_pool(name="xt", bufs=1))
    # separate P^T rings per exp engine (see SCH_PAIRS)
    pt_pool_s = ctx.enter_context(tc.tile_pool(name="pts", bufs=6))
    pt_pool_v = ctx.enter_context(tc.tile_pool(name="ptv", bufs=6))
    ev_pool = ctx.enter_context(tc.tile_pool(name="ev", bufs=6))
    nrm_pool = ctx.enter_context(tc.tile_pool(name="nrm", bufs=2))
    mm_ps = ctx.enter_context(tc.tile_pool(name="mmps", bufs=2, space="PSUM"))
    s_ps = ctx.enter_context(tc.tile_pool(name="sps", bufs=2, space="PSUM"))
    o_ps = ctx.enter_context(tc.tile_pool(name="ops", bufs=2, space="PSUM"))

    # ---- resident x^T (bf16, 4 MB, one [128, 8, 2048] tensor) + weights,
    # pre-cast to bf16 on the host. DMA queue plan (per-engine streams are
    # in-order, so issue order = need order):
    #   sync:   x0, bq, x1, x3   (x0 gates the first Q/K chain)
    #   scalar: wq, wk, wv, x2   (weights first; the x2 descriptor retires
    #                             before the first scalar compute is due)
    #   gpsimd: (wfc later, after pass A)
    xt_all = xt_pool.tile([128, EC, S], DT_X, name="xt", tag="xt")
    xTv = xT_d.rearrange("(ec p) s -> p ec s", p=128)
    wq_all = sbW.tile([128, EC, DG], DT_X, name="wq", tag="wq")
    wk_all = sbW.tile([128, EC, DG], DT_X, name="wk", tag="wk")
    wv_all = sbW.tile([128, EC, DG], DT_X, name="wv", tag="wv")
    def load_w(wt, wd, half, eng=None):
        (eng or nc.scalar).dma_start(
            wt[:, half * 4:(half + 1) * 4, :],
            wd.rearrange("(ec p) d -> p ec d",
                         p=128)[:, half * 4:(half + 1) * 4, :])

    # ---- PE warm-up: the HAM clock gate starts at half clock and only
    # ramps after ~3.4us of sustained matmul activity. Burn the DMA
    # lead-in (PE is idle anyway) on zero matmuls so the gate is warm
    # when real operands land. ----
    wu_sb = sbW.tile([128, 512], DT_X, name="wu", tag="wu")
    nc.vector.memset(wu_sb[:], 0.0)
    for i in range(N_WARMUP):
        wu_ps = o_ps.tile([65, 512], F32, name="wu_ps", tag="po")
        nc.tensor.matmul(wu_ps[:], wu_sb[:, 0:65], wu_sb[:],
                         start=True, stop=True)

    # Transfers within one queue serialize; across queues they share the
    # HBM bandwidth concurrently. Only sync + scalar are HWDGE (full
    # bandwidth) queues. Order pieces by first use, with x chunk 0 split
    # across both queues so the first Q/K chain unblocks earliest:
    #   sync:   x0[ec0-3], bq, x0[ec4-7], x1, x3, (wfc after pass A)
    #   scalar: wq, wk, wv, x2
    nc.sync.dma_start(xt_all[:, 0:4, 0:512], xTv[:, 0:4, 0:512])
    load_w(wq_all, wq_d, 0)
    load_w(wq_all, wq_d, 1)
    # bq as [128, 4]: column dt holds the 128 bias values of head-pair dt
    # (f32 so it can be the scalar-activation bias on the f32 psum)
    bq_t = sbW.tile([128, DTL], F32, name="bq", tag="bq")
    nc.sync.dma_start(bq_t[:], bq_d.rearrange("o (a p) -> (o p) a", p=128))
    nc.sync.dma_start(xt_all[:, 4:8, 0:512], xTv[:, 4:8, 0:512])
    load_w(wk_all, wk_d, 0)
    load_w(wk_all, wk_d, 1)
    load_w(wv_all, wv_d, 0)
    load_w(wv_all, wv_d, 1)
    # x1/x2/x3 behind the weights on the scalar queue: they never
    # contend with the weight transfers the early chains wait on, and
    # the scalar compute they head-of-line block (g0's first exps and
    # V evacuations) has tens of microseconds of pipeline slack
    nc.scalar.dma_start(xt_all[:, :, 512:1024], xTv[:, :, 512:1024])
    nc.scalar.dma_start(xt_all[:, :, 1024:1536], xTv[:, :, 1024:1536])
    nc.scalar.dma_start(xt_all[:, :, 1536:2048], xTv[:, :, 1536:2048])
    xt_res = [[xt_all[:, ec, sc * 512:(sc + 1) * 512] for sc in range(SC)]
              for ec in range(EC)]
    wq_t = [wq_all[:, ec, :] for ec in range(EC)]
    wk_t = [wk_all[:, ec, :] for ec in range(EC)]
    wv_t = [wv_all[:, ec, :] for ec in range(EC)]

    # ---- persistent activations (per-s-chunk tiles so attention's
    # dependencies are fine-grained and can overlap the projection) ----
    QT = [[sbP.tile([128, 512], DT_ATTN, name=f"QT{i}_{s}", tag=f"QT{i}_{s}")
           for s in range(SC)] for i in range(DTL)]
    KT = [[sbP.tile([128, 512], DT_ATTN, name=f"KT{i}_{s}", tag=f"KT{i}_{s}")
           for s in range(SC)] for i in range(DTL)]
    # V_aug in fp8 (values pre-scaled x32 on host; wfc carries 1/32), packed
    # as DoubleRow k-tile pairs: VA2[tp][:, ko, h*66+m] = V_aug for k-tile
    # 2*tp+ko, head h, column m (64 V cols + ones col at m=64; m=65 pad).
    # The ko stride (528) is a multiple of 16 as DoubleRow LDWEIGHTS needs.
    VA2 = [sbP.tile([128, 2, 8 * 66], FP8, name=f"VA2{i}", tag=f"VA2{i}")
           for i in range(ST // 2)]
    OT = [[sbP.tile([128, 512], DT_PROJ, name=f"OT{i}_{s}", tag=f"OT{i}_{s}")
           for s in range(SC)] for i in range(DTL)]

    # ones columns of V_aug (col 64 of each head's 66-col block)
    for tp in range(ST // 2):
        va4 = VA2[tp].rearrange("p ko (h c) -> p ko h c", c=66)
        nc.vector.memset(va4[:, :, :, 64:65], 1.0)
    # f32 ones row for the tail's PE-side partition broadcast
    ones_f = sbW.tile([1, 64], F32, name="ones_f", tag="ones_f")
    nc.vector.memset(ones_f[:], 1.0)

    # ---- projection pieces ----
    def emit_qk(dt_i, sc):
        """Q^T (with bias) and K^T for one head-pair tile, one s-chunk."""
        dsl = slice(dt_i * 128, (dt_i + 1) * 128)
        pq = mm_ps.tile([128, 512], F32, name="pq", tag="mm")
        for ec in range(EC):
            nc.tensor.matmul(pq[:], wq_t[ec][:, dsl], xt_res[ec][sc][:],
                             start=(ec == 0), stop=(ec == EC - 1))
        nc.vector.tensor_scalar_add(QT[dt_i][sc][:], pq[:],
                                    bq_t[:, dt_i:dt_i + 1])
        pk = mm_ps.tile([128, 512], F32, name="pk", tag="mm")
        for ec in range(EC):
            nc.tensor.matmul(pk[:], wk_t[ec][:, dsl], xt_res[ec][sc][:],
                             start=(ec == 0), stop=(ec == EC - 1))
        nc.vector.tensor_copy(KT[dt_i][sc][:], pk[:])

    def emit_v(sc):
        for st_l in range(4):
            st = sc * 4 + st_l
            ssl = slice(st_l * 128, (st_l + 1) * 128)
            pv = mm_ps.tile([128, 512], F32, name="pv", tag="mm")
            for ec in range(EC):
                nc.tensor.matmul(pv[:], xt_res[ec][sc][:, ssl], wv_t[ec][:],
                                 start=(ec == 0), stop=(ec == EC - 1))
            va4 = VA2[st // 2].rearrange("p ko (h c) -> p ko h c", c=66)
            pv3 = pv.rearrange("p (h d) -> p h d", d=64)
            # pass A only; ScalarE is idle there while the DVE carries
            # the K^T/Q^T evacuations and g0's Schraudolph tiles
            nc.scalar.copy(va4[:, st % 2, :, 0:64], pv3[:])

    def emit_fc(sc, tail=False):
        s0 = sc * 512
        big = None
        for nt in range(NT):
            nsl = slice(nt * 128, (nt + 1) * 128)
            if tail and nt % 4 >= 2:
                # the score psums are idle in the tail: borrow both
                # halves of each so the FC accumulation ring is 6 deep
                # and six dt0-2 partial chains prefill during the last
                # normalize
                if nt % 2 == 0:
                    big = s_ps.tile([128, 1024], F32, name="pyb", tag="ps")
                py = big[:, (nt % 2) * 512:(nt % 2 + 1) * 512]
            else:
                py = mm_ps.tile([128, 512], F32, name="py", tag="mm")[:]
            for dt_i in range(DTL):
                nc.tensor.matmul(py, wfc_t[dt_i][:, nsl],
                                 OT[dt_i][sc][:],
                                 start=(dt_i == 0), stop=(dt_i == DTL - 1))
            yv = ev_pool.tile([128, 512], F32, name="yv", tag="yv")
            # split the psum evacuation across both exp engines so the
            # psum ring recycles twice as fast
            if nt % 2 == 0:
                nc.scalar.copy(yv[:], py)
            else:
                nc.vector.tensor_copy(yv[:], py)
            nc.sync.dma_start(yT_d[nt * 128:(nt + 1) * 128, s0:s0 + 512],
                              yv[:])

    def attn_group(hp, qc):
        """Attention for one (head-pair, q-chunk) group, split-emittable.

        Returns (pairs, finish): pairs(lo, hi) emits S/exp/PV for k-tile
        pairs [lo, hi); finish() flushes the PV pipeline and normalizes.
        Callers must emit pairs in order and only after the KT/VA2 tiles
        for that k-range have been emitted (program-order dependency).
        """
        po = [o_ps.tile([65, 512], F32, name=f"po{p}", tag="po")
              for p in range(2)]

        def emit_pv(tp, pt2):
            # DoubleRow PV: one fp8 matmul per head covers k-tile pair
            # tp (256 keys). Stationary = VA2 pair slice [128, 2, 65];
            # moving = P^T pair [128, 2, 512] (ko-strided AP).
            pv4 = pt2.rearrange("p (ko h q) -> p h ko q", ko=2, h=2)
            va4 = VA2[tp].rearrange("p ko (h c) -> p ko h c", c=66)
            for p in range(2):
                h_l = hp * 2 + p
                nc.tensor.matmul(po[p][:],
                                 va4[:, :, h_l, 0:65],
                                 pv4[:, p],
                                 start=(tp == 0),
                                 stop=(tp == ST // 2 - 1),
                                 perf_mode=DR)

        # software-pipelined k-tile pairs: S/exp of pair tp are emitted
        # ahead of the PV of pair tp-3/-4; PVs are flushed two pairs at
        # a time so the 64<->128 tiling-mode switches halve.
        pend = []  # [(tp, pt2)] awaiting PV

        sch_pairs = SCH_PAIRS_FC if hp == DTL - 1 else SCH_PAIRS

        def pairs(lo, hi):
            for tp in range(lo, hi):
                # pt2 free-dim layout: ko*1024 + h*512 + q -> the exp
                # engines write contiguous [128, 1024] tiles per k-tile.
                sch = tp in sch_pairs
                pool = pt_pool_v if sch else pt_pool_s
                pt2 = pool.tile([128, 2048], FP8, name="ptt", tag="ptt")
                pt2v = pt2.rearrange("p (ko hq) -> p ko hq", ko=2)
                pt2u = pt2.bitcast(U8).rearrange("p (ko hq) -> p ko hq",
                                                 ko=2)
                for ko in range(2):
                    kt = 2 * tp + ko
                    ps_t = s_ps.tile([128, 1024], F32, name="ps", tag="ps")
                    for p in range(2):
                        psl = slice(p * 64, (p + 1) * 64)
                        nc.tensor.matmul(ps_t[:, p * 512:(p + 1) * 512],
                                         KT[hp][kt // 4][psl, (kt % 4) * 128:
                                                         (kt % 4) * 128 + 128],
                                         QT[hp][qc][psl, :],
                                         start=True, stop=True)
                    if sch:
                        nc.vector.tensor_scalar(pt2u[:, ko], ps_t[:],
                                                SCH_A, SCH_B,
                                                mybir.AluOpType.mult,
                                                mybir.AluOpType.add)
                    else:
                        nc.scalar.activation(pt2v[:, ko], ps_t[:],
                                             mybir.ActivationFunctionType.Exp)
                pend.append((tp, pt2))
                if len(pend) >= 4:
                    emit_pv(*pend.pop(0))
                    emit_pv(*pend.pop(0))

        def finish(tail=False):
            while pend:
                emit_pv(*pend.pop(0))
            _normalize(tail=tail)

        def _normalize(tail=False):
            # Evacuate both PV psums (releases the po banks), batch the
            # two heads' Z rows into one reciprocal, broadcast + multiply
            # on GpSimd. In the tail (nothing left to overlap) keep the
            # chain on the drained Scalar/Vector engines for min latency.
            # single [65, 1024] tile: head p in columns p*512:(p+1)*512,
            # evacuated by both exp engines in parallel, one Z-row DMA
            posb2 = nrm_pool.tile([65, 1024], F32, name="posb", tag="posb")
            posb = [posb2[:, 0:512], posb2[:, 512:1024]]
            # non-tail: both evacuations on the DVE — a posb copy in the
            # scalar stream sits behind the group's pending exps and
            # stalls the whole normalize chain at every group boundary
            if tail:
                nc.scalar.copy(posb[0], po[0][:])
            else:
                nc.vector.tensor_copy(posb[0], po[0][:])
            nc.vector.tensor_copy(posb[1], po[1][:])
            # custom DVE ops and partition_broadcast read the tensor's
            # partition 0 regardless of AP offset -> move Z via DMA first
            rz2 = nrm_pool.tile([1, 1024], F32, name="rz2", tag="rz2")
            nc.sync.dma_start(rz2[:], posb2[64:65, :])
            rzr = nrm_pool.tile([1, 1024], F32, name="rzr", tag="rzr")
            nc.vector.reciprocal_approx_fast(rzr[:], rz2[:])
            if tail:
                # the PE is idle in the tail and the gpsimd wake + drain
                # latency (~4us) would sit on the critical path: do the
                # partition broadcast as ones[1,64].T @ rz[1,512] into
                # the freed po psum banks instead
                rzt = [o_ps.tile([65, 512], F32, name=f"rzbp{p}",
                                 tag="po") for p in range(2)]
                rzb = [t[0:64, :] for t in rzt]
                for p in range(2):
                    nc.tensor.matmul(rzb[p], ones_f[:],
                                     rzr[0:1, p * 512:(p + 1) * 512],
                                     start=True, stop=True)
            else:
                # broadcasts on GpSimd (its ONLY op class: switching
                # between custom and standard gpsimd ops reloads the Q7
                # library at ~5.6us per switch); multiplies on the DVE
                rzt = [nrm_pool.tile([64, 512], F32, name=f"rzb{p}",
                                     tag=f"rzb{p}") for p in range(2)]
                rzb = [t[:] for t in rzt]
                for p in range(2):
                    nc.gpsimd.partition_broadcast(
                        rzb[p], rzr[0:1, p * 512:(p + 1) * 512])
            tmp = nrm_pool.tile([64, 512], DT_PROJ, name="otmp", tag="otmp")
            nc.vector.tensor_mul(OT[hp][qc][0:64, :], posb[0][0:64, :],
                                 rzb[0])
            nc.vector.tensor_mul(tmp[:], posb[1][0:64, :], rzb[1])
            # engines cannot shift partitions; DMA moves rows 0:64 into
            # OT rows 64:128.
            nc.sync.dma_start(OT[hp][qc][64:128, :], tmp[:])

        return pairs, finish

    def emit_attn(hp, qc):
        pairs, finish = attn_group(hp, qc)
        pairs(0, ST // 2)
        finish()

    # ---- pass A: V (all heads) + Q/K for head-pair 0, with head-pair
    # 0's first attention group interleaved so the exp engines start as
    # soon as the first chunk's Q/K land (S/exp of pairs [lo,hi) only
    # needs KT; the PV of pair tp is emitted a few pairs later, by which
    # point the V chains covering it are in program order). ----
    g0_pairs, g0_finish = attn_group(0, 0)
    for sc in range(SC):
        emit_qk(0, sc)
        emit_v(sc)
        g0_pairs(2 * sc, 2 * sc + 2)
    emit_qk(1, 0)

    # ---- attention groups, score-primed across boundaries ----
    # Each group's first two score pairs are emitted BEFORE the previous
    # group's PV flush + normalize, so the exp pipeline never drains at
    # a group boundary. Deferred projections are spread one chain per
    # group (their psum evacuations then sit between group workloads in
    # the in-order DVE queue instead of piling up behind a window); the
    # hp3 groups carry the FC of the previous q-chunk instead.
    wfc_t = []
    prev_fin = g0_finish
    seq = [(0, qc) for qc in range(1, SC)] + \
          [(hp, qc) for hp in range(1, DTL) for qc in range(SC)]
    for hp, qc in seq:
        pairs, fin = attn_group(hp, qc)
        pairs(0, 2)
        prev_fin()
        pairs(2, 4)
        if hp == DTL - 1 and qc >= 1:
            emit_fc(qc - 1)
        pairs(4, ST // 2)
        if hp < DTL - 1 and qc >= (1 if hp == 0 else 0):
            emit_qk(hp + 1, qc)
        prev_fin = fin
        if (hp, qc) == (0, SC - 1):
            # wfc deferred past pass A: first FC use is in the hp3
            # window, and these 2MB would otherwise crowd the DMA-paced
            # startup.
            for dt_i in range(DTL):
                t = sbW.tile([128, E], DT_PROJ, name=f"wfc{dt_i}",
                             tag=f"wfc{dt_i}")
                nc.sync.dma_start(t[:], wfc_d[dt_i * 128:(dt_i + 1) * 128, :])
                wfc_t.append(t)
    prev_fin(tail=True)
    emit_fc(SC - 1, tail=True)

_CACHE = {}


def _build():
    if "nc" in _CACHE:
        return _CACHE["nc"]
    nc = bacc.Bacc("TRN2", target_bir_lowering=False, debug=False)
    io = {
        "xT": nc.dram_tensor("xT", [E, S], BF16, kind="ExternalInput").ap(),
        "wq": nc.dram_tensor("wq", [E, DG], BF16, kind="ExternalInput").ap(),
        "wk": nc.dram_tensor("wk", [E, DG], BF16, kind="ExternalInput").ap(),
        "wv": nc.dram_tensor("wv", [E, DG], BF16, kind="ExternalInput").ap(),
        "bq": nc.dram_tensor("bq", [1, DG], F32, kind="ExternalInput").ap(),
        "wfc": nc.dram_tensor("wfc", [DG, E], BF16,
                              kind="ExternalInput").ap(),
        "yT": nc.dram_tensor("yT", [E, S], F32, kind="ExternalOutput").ap(),
    }
    with tile.TileContext(nc) as tc:
        _emit(tc, io)
    nc.compile()
    _CACHE["nc"] = nc
    return nc


def make_in_maps(x, w_qkv, b_qkv, w_fc):
    """Host-side sharding: returns per-core input dicts (core = b*G + g)."""
    import ml_dtypes
    x = np.asarray(x, dtype=np.float32)
    w_qkv = np.asarray(w_qkv, dtype=np.float32)
    b_qkv = np.asarray(b_qkv, dtype=np.float32)
    w_fc = np.asarray(w_fc, dtype=np.float32)
    npdt = ml_dtypes.bfloat16
    in_maps = []
    for b in range(B):
        xTb = np.ascontiguousarray(x[b].T).astype(npdt)
        for g in range(G):
            gs = slice(g * DG, (g + 1) * DG)
            in_maps.append({
                "xT": xTb,
                "wq": np.ascontiguousarray(
                    w_qkv[:, 0 * E:1 * E][:, gs] * (1.0 / np.sqrt(D))
                ).astype(npdt),
                "wk": np.ascontiguousarray(w_qkv[:, 1 * E:2 * E][:, gs]).astype(npdt),
                # V path scaled x32 so the fp8(e4m3) V_aug values sit in the
                # normal range; wfc carries the 1/32 back out.
                "wv": np.ascontiguousarray(
                    w_qkv[:, 2 * E:3 * E][:, gs] * 32.0).astype(npdt),
                "bq": np.ascontiguousarray(
                    b_qkv[0 * E:1 * E][gs][None, :] * (1.0 / np.sqrt(D))
                ).astype(np.float32),
                "wfc": np.ascontiguousarray(
                    w_fc[gs, :] * (1.0 / 32.0)).astype(npdt),
            })
    return in_maps


def gather(results, b_qkv, w_fc, b_fc):
    """Host-side unshard: sum group partials, transpose, add const bias."""
    b_qkv = np.asarray(b_qkv, dtype=np.float32)
    w_fc = np.asarray(w_fc, dtype=np.float32)
    b_fc = np.asarray(b_fc, dtype=np.float32)
    cbias = (b_qkv[2 * E:3 * E].astype(np.float64) @ w_fc.astype(np.float64)
             + b_fc.astype(np.float64)).astype(np.float32)
    y = np.empty((B, S, E), np.float32)
    for b in range(B):
        yT = results[b * G]["yT"] + results[b * G + 1]["yT"]
        y[b] = yT.T + cbias[None, :]
    return y


def kernel(x, w_qkv, b_qkv, w_fc, b_fc, _trace=False, _tmpdir=None):
    nc = _build()
    in_maps = make_in_maps(x, w_qkv, b_qkv, w_fc)
    res = run_bass_kernel_spmd(nc, in_maps, list(range(NCORES)),
                               trace=_trace, tmpdir=_tmpdir)
    y = gather(res.results, b_qkv, w_fc, b_fc)
    kernel.last_exec_time_ns = res.exec_time_ns
    kernel.last_res = res
    return y
